# revision 8
# baseline (speedup 1.0000x reference)
"""Bass/Trainium2 kernel for nn_KeypointPPF_EdgeConv.

Strategy (8 NeuronCores, data-parallel over batch B=8):
  Host (numpy): fold BatchNorms into affine weights; compute PPF features and
  the tiny stage-A MLPs (pos_encoder, ppf layer1) on host; also compute the
  per-point e1 contribution cd = kpt @ A_cd.T on host. Device does 8 bf16
  matmul passes per edge (the PE floor for exact bf16):
    e1:  psum1 = Wnf@nfT + Wext@poshT_ext          (2 passes/chunk)
    y1  = relu(psum1 + b1)                          (ACT, bias fused)
    e2:  psum2 = W2a@y1a + W2b@y1b                  (2 passes/chunk)
    out = reduce_max over k                         (DVE)
  The per-point cd term rides FREE inside the posh pass: posh has only 96
  real contraction rows; rows 96:128 of the moving tile hold a constant
  one-hot point-indicator (col p*16+k -> row 96+p), and rows 96:128 of the
  per-(group,chunk) stationary hold cd[pt, out]. This removes the 2
  broadcast passes the old kernel spent per group.

Edge order: group g = 32 points x 16 neighbors (pt-major: f = pt*16 + k).
Loads are batched 4 groups (2048 edges) per DMA; e2(g-1) is emitted between
e1(g) and e1(g+1) to keep PE busy while ACT computes y1(g).
"""

import sys

sys.path.insert(0, "/opt/trn_rl_repo")

import numpy as np
import ml_dtypes

import concourse.bass as bass
import concourse.bacc as bacc
import concourse.mybir as mybir
import concourse.tile as tile
from concourse.bass_utils import run_bass_kernel_spmd

B, N, K, C, COUT = 8, 4096, 16, 128, 256
G = 128          # groups per core
PTS = 32         # points per group
F = PTS * K      # 512 edges per group
T = 32           # load tiles (4 groups each)
FT = 4 * F       # 2048 edges per load tile
BN_EPS = 1e-5
BF16 = mybir.dt.bfloat16
F32 = mybir.dt.float32
NPBF16 = ml_dtypes.bfloat16

_CACHE = {}


def build_nc():
    nc = bacc.Bacc("TRN2", target_bir_lowering=False, debug=False)
    nfT = nc.declare_dram_parameter("nfT", [T, C, FT], BF16, isOutput=False)
    poshT = nc.declare_dram_parameter("poshT", [T, 96, FT], BF16, isOutput=False)
    blob = nc.declare_dram_parameter("blob", [T, 128, 1024], BF16, isOutput=False)
    ident = nc.declare_dram_parameter("ident", [32, FT], BF16, isOutput=False)
    w_nf = nc.declare_dram_parameter("w_nf", [C, COUT], BF16, isOutput=False)
    w_e2a = nc.declare_dram_parameter("w_e2a", [128, COUT], BF16, isOutput=False)
    w_e2b = nc.declare_dram_parameter("w_e2b", [128, COUT], BF16, isOutput=False)
    bias1 = nc.declare_dram_parameter("bias1", [128, 2], F32, isOutput=False)
    bias2 = nc.declare_dram_parameter("bias2", [128, 2], F32, isOutput=False)
    out = nc.declare_dram_parameter("out", [COUT, N], F32, isOutput=True)

    with tile.TileContext(nc) as tc:
        with (
            tc.tile_pool(name="consts", bufs=1) as cpool,
            tc.tile_pool(name="posh", bufs=1) as phpool,
            tc.tile_pool(name="loads", bufs=3) as lpool,
            tc.tile_pool(name="y1", bufs=2) as ypool,
            tc.tile_pool(name="outT", bufs=1) as opool,
            tc.tile_pool(name="psum1", bufs=2, space="PSUM") as p1pool,
            tc.tile_pool(name="psum2", bufs=2, space="PSUM") as p2pool,
        ):
            # resident constants
            wnf_sb = cpool.tile([C, COUT], BF16, tag="wnf")
            nc.sync.dma_start(wnf_sb[:], w_nf[:])
            we2a_sb = cpool.tile([128, COUT], BF16, tag="we2a")
            nc.sync.dma_start(we2a_sb[:], w_e2a[:])
            we2b_sb = cpool.tile([128, COUT], BF16, tag="we2b")
            nc.sync.dma_start(we2b_sb[:], w_e2b[:])
            b1_sb = cpool.tile([128, 2], F32, tag="b1")
            nc.sync.dma_start(b1_sb[:], bias1[:])
            b2_sb = cpool.tile([128, 2], F32, tag="b2")
            nc.sync.dma_start(b2_sb[:], bias2[:])

            # 3 fixed posh tiles; rows 96:128 pre-filled once with the
            # constant one-hot point-indicator pattern
            posh_tiles = []
            for i in range(3):
                pt_sb = phpool.tile([128, FT], BF16, tag=f"posh{i}")
                nc.sync.dma_start(pt_sb[96:128, :], ident[:])
                posh_tiles.append(pt_sb)

            outT0 = opool.tile([128, N], F32, tag="outT0")
            outT1 = opool.tile([128, N], F32, tag="outT1")
            outTs = [outT0, outT1]

            def emit_e2(g, y1s):
                for m in range(2):
                    mm = slice(m * 128, (m + 1) * 128)
                    psum2 = p2pool.tile([128, F], F32, tag=f"p2_{m}")
                    nc.tensor.matmul(
                        psum2[:], we2a_sb[:, mm], y1s[0][:], start=True, stop=False
                    )
                    nc.tensor.matmul(
                        psum2[:], we2b_sb[:, mm], y1s[1][:], start=False, stop=True
                    )
                    nc.vector.tensor_reduce(
                        outTs[m][:, g * PTS:(g + 1) * PTS],
                        psum2[:].rearrange("p (a b) -> p a b", b=K),
                        axis=mybir.AxisListType.X,
                        op=mybir.AluOpType.max,
                    )
                # flush finished 128-col output slices so the final relu +
                # store overlap with remaining compute instead of tailing
                if (g + 1) % 4 == 0:
                    sl = slice((g + 1) * PTS - 128, (g + 1) * PTS)
                    for m in range(2):
                        nc.scalar.activation(
                            outTs[m][:, sl],
                            outTs[m][:, sl],
                            mybir.ActivationFunctionType.Relu,
                            bias=b2_sb[:, m:m + 1],
                        )
                        nc.sync.dma_start(
                            out[m * 128:(m + 1) * 128, sl], outTs[m][:, sl]
                        )

            # PE warm-up: junk matmuls ramp the tensor-engine pstate while the
            # first input DMAs are in flight
            warm = p1pool.tile([128, F], F32, tag="p1_0")
            for _ in range(10):
                nc.tensor.matmul(
                    warm[:, 0:COUT], we2a_sb[:, 0:128], we2a_sb[:],
                    start=True, stop=True,
                )

            def emit_e1(g, nf_ap, posh_ap, blob_sb, boff):
                y1s = []
                for m in range(2):
                    mm = slice(m * 128, (m + 1) * 128)
                    psum1 = p1pool.tile([128, F], F32, tag=f"p1_{m}")
                    nc.tensor.matmul(
                        psum1[:], wnf_sb[:, mm], nf_ap, start=True, stop=False,
                    )
                    nc.tensor.matmul(
                        psum1[:],
                        blob_sb[:, boff + m * 128:boff + (m + 1) * 128],
                        posh_ap,
                        start=False, stop=True,
                    )
                    y1 = ypool.tile([128, F], BF16, tag=f"y1_{m}")
                    nc.scalar.activation(
                        y1[:], psum1[:], mybir.ActivationFunctionType.Relu,
                        bias=b1_sb[:, m:m + 1],
                    )
                    y1s.append(y1)
                return y1s

            prev = None
            for t in range(T):
                nf_sb = lpool.tile([C, FT], BF16, tag="nfT")
                # 1-elem memset absorbs the WAR wait on the Pool engine so the
                # DMA itself carries <=1 sync wait (walrus DIRECT2D limit)
                nc.gpsimd.memset(nf_sb[0:1, 0:1], 0)
                posh_sb = posh_tiles[t % 3]
                nc.gpsimd.memset(posh_sb[0:1, 0:1], 0)
                blob_sb = lpool.tile([128, 1024], BF16, tag="blob")
                nc.gpsimd.memset(blob_sb[0:1, 0:1], 0)
                if t == 0:
                    # sub-chunk the first tile's loads per group so e1(g)
                    # waits only on its own ~288 KiB, not the full 1.2 MiB
                    for j in range(4):
                        cf = slice(j * F, (j + 1) * F)
                        nc.gpsimd.dma_start(nf_sb[:, cf], nfT[0][:, cf])
                        nc.gpsimd.dma_start(posh_sb[0:96, cf], poshT[0][:, cf])
                        nc.gpsimd.dma_start(
                            blob_sb[:, j * 256:(j + 1) * 256],
                            blob[0][:, j * 256:(j + 1) * 256],
                        )
                else:
                    nc.gpsimd.dma_start(nf_sb[:], nfT[t])
                    nc.gpsimd.dma_start(posh_sb[0:96, :], poshT[t])
                    nc.gpsimd.dma_start(blob_sb[:], blob[t])

                for j in range(4):
                    g = 4 * t + j
                    cols = slice(j * F, (j + 1) * F)
                    y1s = emit_e1(
                        g, nf_sb[:, cols], posh_sb[:, cols], blob_sb, j * 256
                    )
                    if prev is not None:
                        emit_e2(*prev)
                    prev = (g, y1s)
            emit_e2(*prev)
    nc.compile()
    return nc


def _prep(inputs):
    f32 = np.float32
    e1_w = inputs["e1_w"].astype(f32)
    s1 = inputs["e1_g"] / np.sqrt(inputs["e1_v"] + BN_EPS)
    t1 = inputs["e1_beta"] - inputs["e1_m"] * s1
    s2 = inputs["e2_g"] / np.sqrt(inputs["e2_v"] + BN_EPS)
    t2 = inputs["e2_beta"] - inputs["e2_m"] * s2
    sp = inputs["pos_g"] / np.sqrt(inputs["pos_v"] + BN_EPS)
    tp = inputs["pos_beta"] - inputs["pos_m"] * sp
    sf = inputs["ppf_g"] / np.sqrt(inputs["ppf_v"] + BN_EPS)
    tf = inputs["ppf_beta"] - inputs["ppf_m"] * sf

    W_c, W_d = e1_w[:, 0:128], e1_w[:, 128:256]
    W_p, W_q = e1_w[:, 256:320], e1_w[:, 320:384]

    A_nf = s1[:, None] * W_d                         # [256,128]
    A_cd = s1[:, None] * (W_c - W_d)                 # [256,128]
    A_pos = s1[:, None] * W_q                        # [256,64]
    A_h = (s1[:, None] * W_p) @ inputs["ppf_w2"]     # [256,32]
    b1p = s1 * (inputs["e1_b"] + W_p @ inputs["ppf_b2"]) + t1
    A_posh = np.concatenate([A_pos, A_h], axis=1)    # [256,96]
    W2p = s2[:, None] * inputs["e2_w"]
    b2p = s2 * inputs["e2_b"] + t2

    # host stage-A features
    kx = inputs["kpt_xyz"]                            # [B,N,3]
    nx = inputs["neighbor_xyz"]                       # [B,N,K,3]
    nn = inputs["neighbor_normals"]
    rel = nx - kx[:, :, None, :]
    kn = nn.mean(axis=2)
    kn = kn / np.maximum(np.linalg.norm(kn, axis=-1, keepdims=True), 1e-12)
    n1 = kn[:, :, None, :]
    d_norm = np.linalg.norm(rel, axis=-1, keepdims=True)
    d = rel / (d_norm + 1e-8)
    alpha = np.clip(np.sum(n1 * d, -1, keepdims=True), -1.0, 1.0)
    phi = np.clip(np.sum(nn * d, -1, keepdims=True), -1.0, 1.0)
    theta = np.clip(np.sum(n1 * nn, -1, keepdims=True), -1.0, 1.0)
    ppf = np.concatenate([d_norm, alpha, phi, theta], -1)  # [B,N,K,4]

    Wpe = (inputs["pos_w"] * sp[:, None]).T           # [3,64]
    cpe = sp * inputs["pos_b"] + tp
    W1e = (inputs["ppf_w1"] * sf[:, None]).T          # [4,32]
    c1e = sf * inputs["ppf_b1"] + tf
    pos_enc = np.maximum(rel @ Wpe + cpe, 0.0)        # [B,N,K,64]
    h = np.maximum(ppf @ W1e + c1e, 0.0)              # [B,N,K,32]
    posh = np.concatenate([pos_enc, h], axis=-1).astype(f32)  # [B,N,K,96]

    # one-hot point-indicator: row p, col lp*16+k -> 1 iff lp%32 == p
    ident = np.zeros((32, FT), dtype=NPBF16)
    lp = (np.arange(FT) // K) % PTS
    ident[lp, np.arange(FT)] = 1

    A_poshT = np.ascontiguousarray(A_posh.T).astype(f32)  # [96,256]

    weights = {
        "w_nf": np.ascontiguousarray(A_nf.T).astype(NPBF16),
        "w_e2a": np.ascontiguousarray(W2p.T[0:128]).astype(NPBF16),
        "w_e2b": np.ascontiguousarray(W2p.T[128:256]).astype(NPBF16),
        "bias1": np.ascontiguousarray(b1p.astype(f32).reshape(2, 128).T),
        "bias2": np.ascontiguousarray(b2p.astype(f32).reshape(2, 128).T),
        "ident": ident,
    }

    in_maps = []
    for b in range(B):
        # [N,K,C] -> tiles [T, 2048, C] -> [T, C, 2048]
        nf_g = (
            inputs["neighbor_feature"][b]
            .reshape(T, FT, C)
            .transpose(0, 2, 1)
        )
        posh_g = posh[b].reshape(T, FT, 96).transpose(0, 2, 1)
        # per-point e1 contribution cd[n, out] = kpt[n] @ A_cd.T
        cd = inputs["kpt_feature"][b].astype(f32) @ A_cd.T  # [N,256]
        # stationary blob [T, 128, 4, 2, 128]:
        #   rows 0:96  = A_poshT[:, m*128:(m+1)*128]  (replicated per group)
        #   rows 96:128 = cd[g*32:(g+1)*32, m*128:(m+1)*128]
        blob6 = np.empty((T, 128, 4, 2, 128), dtype=NPBF16)
        ap = A_poshT.reshape(96, 2, 128).astype(NPBF16)      # [96, m, 128]
        blob6[:, 0:96] = ap[None, :, None, :, :]
        cd_r = cd.reshape(T, 4, PTS, 2, 128).transpose(0, 2, 1, 3, 4)
        blob6[:, 96:128] = cd_r.astype(NPBF16)
        m = {
            "nfT": np.ascontiguousarray(nf_g).astype(NPBF16),
            "poshT": np.ascontiguousarray(posh_g).astype(NPBF16),
            "blob": np.ascontiguousarray(blob6.reshape(T, 128, 1024)),
        }
        m.update(weights)
        in_maps.append(m)
    return in_maps


def kernel(trace=False, **inputs):
    if "nc" not in _CACHE:
        _CACHE["nc"] = build_nc()
    nc = _CACHE["nc"]
    in_maps = _prep(inputs)
    res = run_bass_kernel_spmd(nc, in_maps, list(range(B)), trace=trace)
    out = np.stack([res.results[b]["out"].T for b in range(B)])  # [B,N,COUT]
    _CACHE["last"] = res
    return np.ascontiguousarray(out.astype(np.float32))
